# revision 47
# baseline (speedup 1.0000x reference)
# Trainium2 Bass kernel for dynamic-routing capsule layer (nn_Capsule).
#
# Math (per batch b):
#   u_hat[n,i,j] = sum_d u[n,d] W[d, i*16+j]
#   b=0; for it in 0..2:
#     c = softmax(b, axis=i)
#     o[i,j] = sum_n c[i,n] u_hat[n,i,j]
#     if it<2: o' = l2norm(o); b[i,n] = sum_j o'[i,j] u_hat[n,i,j]
#   out = squash(o)
#
# Key algebraic restructuring (u_hat [B,N,512] never materialized):
#   s[i,d]  = sum_n c[i,n] u[n,d]                  (PE: contract n, u natural)
#   o[i,j]  = (S @ W)[i, i*16+j]                   (PE + mask/group-reduce)
#   vT[d,i] = sum_j W[d,i*16+j] o'[i,j]            (DVE: W * bcast(o'), group-reduce)
#   b[i,n]  = sum_d vT[d,i] u[n,d]                 (PE: contract d, uT transposed)
#
# Tokens are processed in partition-major order n = 32*p + c (contiguous 32KB
# DMA per partition); the order is self-consistent across u/uT/b/e/cT and all
# n-reductions are complete sums, so results are order-invariant.
#
# Big matmuls run in float32r (full PE rate, fp32 rounded to 11 mantissa
# bits => ~1e-4 rel err). All PE inputs are produced as f32r (verifier rule).
#
# Sharding: data-parallel over batch B=32 across 8 cores (4 batches/core),
# W replicated. No collectives.

import numpy as np

N_CORES = 8
B, N, D = 32, 4096, 256
I_CAPS, J_DIM = 32, 16
ROUTINGS = 3
EPS = 1e-7
L2_EPS = 1e-12

F32R = True  # float32r for PE matmuls + transposes (fallback: plain fp32)


def build_nc(b_loc=B // N_CORES, n=N, d=D, enable_asserts=False, reps=1):
    from contextlib import ExitStack

    import concourse.bass as bass  # noqa: F401
    import concourse.tile as tile
    from concourse import bacc, mybir
    from concourse.masks import make_identity
    import bass_rust

    def chain(insts):
        # same-engine ordering edges (no semaphores, no basic blocks):
        # keeps a psum accumulation group's start=True member first without
        # the all-engine cost of tc.tile_critical()
        for a, b2 in zip(insts[1:], insts[:-1]):
            bass_rust.add_dep_helper(a.ins, b2.ins, sync=False,
                                     reason="pack order")

    f32 = mybir.dt.float32
    pe_dt = mybir.dt.float32r if F32R else f32
    AX = mybir.AxisListType
    OP = mybir.AluOpType
    ACTF = mybir.ActivationFunctionType

    NC = n // 128       # chunks of 128 tokens
    DC = d // 128       # d chunks of 128
    NB = n // 512       # token chunks of 512 (psum bank width)
    IJ = I_CAPS * J_DIM  # 512

    nc = bacc.Bacc("TRN2", target_bir_lowering=False, debug=False,
                   enable_asserts=enable_asserts)
    u_dram = nc.dram_tensor("u", [b_loc, n, d], f32, kind="ExternalInput").ap()
    w_dram = nc.dram_tensor("w", [1, d, IJ], f32, kind="ExternalInput").ap()
    out_dram = nc.dram_tensor("out", [b_loc, I_CAPS, J_DIM], f32,
                              kind="ExternalOutput").ap()

    with tile.TileContext(nc) as tc, ExitStack() as ctx:
        const_pool = ctx.enter_context(tc.tile_pool(name="const", bufs=1))
        u_pool = ctx.enter_context(tc.tile_pool(name="u", bufs=2))
        uT_pool = ctx.enter_context(tc.tile_pool(name="uT", bufs=2))
        cT_pool = ctx.enter_context(tc.tile_pool(name="cT", bufs=2))
        e_pool = ctx.enter_context(tc.tile_pool(name="e", bufs=2))
        small = ctx.enter_context(tc.tile_pool(name="small", bufs=2))
        tiny = ctx.enter_context(tc.tile_pool(name="tiny", bufs=2))
        psum = ctx.enter_context(tc.tile_pool(name="ps", bufs=1, space="PSUM"))

        # ---- constants ----
        ident = const_pool.tile([128, 128], f32, name="ident")
        make_identity(nc, ident[:])
        # f32r-typed identity (produced by a rounding copy => verifier-legal)
        ident_r = const_pool.tile([128, 128], pe_dt, name="ident_r")
        nc.vector.tensor_copy(ident_r[:], ident[:])

        scratch1 = const_pool.tile([2 * I_CAPS, 128], f32, name="scratch1")
        nc.gpsimd.memset(scratch1[:], 1.0)

        npair = 2 if PAIRED else 1
        P = npair * I_CAPS  # partition rows covering one batch group

        # uniform routing weights for iteration 0 (softmax of zeros == 1/I)
        c0 = const_pool.tile([128, P], pe_dt, name="c0")
        scratch2 = small.tile([128, P], f32, tag="om", name="scratch2")
        nc.gpsimd.memset(scratch2[:], 1.0 / I_CAPS)
        nc.vector.tensor_copy(c0[:], scratch2[:])

        # per-partition bias constants for the squash Ln ops
        eps_ap = const_pool.tile([P, 1], f32, name="eps_ap")
        nc.gpsimd.memset(eps_ap[:], EPS)
        eps_half_ap = const_pool.tile([P, 1], f32, name="eps_half_ap")
        nc.gpsimd.memset(eps_half_ap[:], EPS + 0.5)

        # MASK[32*bb + i, e] = 1 if e // J_DIM == i else 0   ([P, 512])
        mask = const_pool.tile([P, IJ], f32, name="mask")
        nc.gpsimd.memset(mask[:], 0.0)
        for bb in range(npair):
            mslice = mask[bb * I_CAPS:(bb + 1) * I_CAPS, :]
            # keep 0 where (e - 16*i - 15) > 0, else fill 1  -> 1 iff e <= 16i+15
            nc.gpsimd.affine_select(
                out=mslice, in_=mslice, compare_op=OP.is_gt, fill=1.0,
                base=-(J_DIM - 1), pattern=[[1, IJ]], channel_multiplier=-J_DIM)
            # keep where (e - 16*i) >= 0, else fill 0 -> 1 iff 16i <= e <= 16i+15
            nc.gpsimd.affine_select(
                out=mslice, in_=mslice, compare_op=OP.is_ge, fill=0.0,
                base=0, pattern=[[1, IJ]], channel_multiplier=-J_DIM)

        # W natural (rounded to f32r by SWDGE cast): w_sb[q, e, f] = W[128e+q, f]
        w_sb = const_pool.tile([128, DC, IJ], pe_dt, name="w_sb")
        nc.gpsimd.dma_start(w_sb[:], w_dram[0].rearrange("(e q) f -> q e f", q=128))

        body = _kernel_body_pair if PAIRED else _kernel_body
        for rep in range(reps):
            body(nc, tc, mybir, b_loc, n, d, NC, DC, NB, IJ, f32, pe_dt,
                 AX, OP, ACTF, u_dram, w_dram, out_dram,
                 u_pool, uT_pool, cT_pool, e_pool, small, tiny, psum,
                 ident, ident_r, scratch1, c0, mask, w_sb, rep,
                 eps_ap, eps_half_ap, chain)

    nc.compile()
    return nc


def _kernel_body(nc, tc, mybir, b_loc, n, d, NC, DC, NB, IJ, f32, pe_dt,
                 AX, OP, ACTF, u_dram, w_dram, out_dram,
                 u_pool, uT_pool, cT_pool, e_pool, small, tiny, psum,
                 ident, ident_r, scratch1, c0, mask, w_sb, rep,
                 eps_ap, eps_half_ap, chain):
    import concourse.bass as bass  # noqa: F401

    if True:
        for b0 in range(b_loc):
            b = b0
            # ---- load u (f32r cast): u_t[p, c, dd] = u[b, 32p + c, dd] ----
            # u load split in 4 so transposes overlap the DMA tail
            u_t = u_pool.tile([128, NC, d], pe_dt, tag="u", name=f"u_{rep}_{b}")
            qn = NC // 8 if NC % 8 == 0 else NC // 4
            for q in range(NC // qn):
                nc.gpsimd.dma_start(
                    u_t[:, q * qn:(q + 1) * qn, :],
                    u_dram[b].rearrange("(p c) dd -> p c dd",
                                        c=NC)[:, q * qn:(q + 1) * qn, :])

            # ---- uT[q, e, 128c+p] = u_t[p, c, 128e+q] via PE transposes.
            # Two 4-transpose bank groups per critical (halves BB overhead);
            # tr bufs=3 keeps a slot free for the interleaved copies. ----
            uT_t = uT_pool.tile([128, DC, n], pe_dt, tag="uT", name=f"uT_{rep}_{b}")
            cp_flip = 0
            for e in range(DC):
                for cg in range(0, NC, 8):
                    nb2 = min(8, NC - cg)
                    # 2-bank psum tile: 8 transposes (one 4-pack group per
                    # bank) evacuated by a single wide copy — halves the
                    # pack->copy semaphore handoffs
                    tr_ps = psum.tile([128, nb2 * 128], pe_dt, tag="tr", bufs=2,
                                      name=f"trps_{rep}_{b}_{e}_{cg}")
                    for g in range(0, nb2, 4):
                        pack = []
                        for k in range(min(4, nb2 - g)):
                            c = cg + g + k
                            kk = g + k
                            pack.append(nc.tensor.matmul(
                                tr_ps[:, kk * 128:(kk + 1) * 128],
                                u_t[:, c, e * 128:(e + 1) * 128],
                                ident_r[:],
                                is_transpose=True,
                                start=(k == 0), stop=(k == min(4, nb2 - g) - 1)))
                        chain(pack)
                    dst = uT_t[:, e, cg * 128:(cg + nb2) * 128]
                    if cp_flip % 2 == 0:
                        nc.vector.tensor_copy(dst, tr_ps[:])
                    else:
                        nc.scalar.copy(dst, tr_ps[:])
                    cp_flip += 1

            cT = None
            for it in range(ROUTINGS):
                # ---- matmul1: s[i, dd] = sum_n c[i, n] u[n, dd] ----
                s_ps = psum.tile([I_CAPS, d], f32, tag="sO", bufs=1,
                                 name=f"sps_{rep}_{b}_{it}")
                for c in range(NC):
                    lhs1 = c0[:] if cT is None else cT[:, c, :]
                    nc.tensor.matmul(s_ps[:], lhs1, u_t[:, c, :],
                                     start=(c == 0), stop=(c == NC - 1))
                s_sb = small.tile([I_CAPS, d], pe_dt, tag="s_sb",
                                  name=f"ssb_{rep}_{b}_{it}")
                nc.vector.tensor_copy(s_sb[:], s_ps[:])

                # ---- sT[q, e*32+i] = s[i, 128e+q] ----
                sT_ps = psum.tile([128, DC * I_CAPS], pe_dt, tag="tr", bufs=2,
                                  name=f"sTps_{rep}_{b}_{it}")
                pack = [nc.tensor.matmul(
                    sT_ps[:, e * I_CAPS:(e + 1) * I_CAPS],
                    s_sb[:, e * 128:(e + 1) * 128],
                    ident_r[0:I_CAPS, 0:I_CAPS],
                    is_transpose=True, start=(e == 0), stop=(e == DC - 1))
                    for e in range(DC)]
                chain(pack)
                sT_sb = small.tile([128, DC * I_CAPS], pe_dt, tag="sT_sb",
                                   name=f"sTsb_{rep}_{b}_{it}")
                nc.scalar.copy(sT_sb[:], sT_ps[:])

                # ---- O_full = S @ W  [32, 512] ----
                o_ps = psum.tile([I_CAPS, IJ], f32, tag="sO", bufs=1,
                                 name=f"Ops_{rep}_{b}_{it}")
                for e in range(DC):
                    nc.tensor.matmul(o_ps[:], sT_sb[:, e * I_CAPS:(e + 1) * I_CAPS],
                                     w_sb[:, e, :],
                                     start=(e == 0), stop=(e == DC - 1))

                # ---- extract o[i, j] = O_full[i, i*16+j] ----
                om_sb = small.tile([I_CAPS, IJ], pe_dt, tag="om",
                                   name=f"om_{rep}_{b}_{it}")
                nc.vector.tensor_mul(om_sb[:], o_ps[:], mask[:])
                om_f = (om_sb[:] if mybir.dt.size(om_sb.dtype) != 4
                        else om_sb[:].bitcast(f32))
                o_sb = tiny.tile([I_CAPS, J_DIM], f32, tag="o", name=f"o_{rep}_{b}_{it}")
                nc.vector.tensor_reduce(
                    o_sb[:], om_f.rearrange("p (i j) -> p j i", j=J_DIM),
                    axis=AX.X, op=OP.add)

                # ---- ||o||^2 per capsule ----
                sq = tiny.tile([I_CAPS, J_DIM], f32, tag="sq", name=f"sq_{rep}_{b}_{it}")
                nrm = tiny.tile([I_CAPS, 1], f32, tag="nrm", name=f"nrm_{rep}_{b}_{it}")
                nc.scalar.activation(sq[:], o_sb[:], ACTF.Square, accum_out=nrm[:])

                if it < ROUTINGS - 1:
                    # ---- l2 normalize scale rr = nrm^-0.5 = exp(-0.5*ln(nrm)).
                    # Ln+Exp live in one ACT table set (unlike Sqrt+Exp), so
                    # this avoids a ~1.3us act-table reload per iteration.
                    # (max(nrm, L2_EPS) dropped: nrm ~ O(1) >> 1e-12 here.)
                    lnm = tiny.tile([I_CAPS, 1], f32, tag="lnm",
                                    name=f"lnm_{rep}_{b}_{it}")
                    nc.scalar.activation(lnm[:], nrm[:], ACTF.Ln)
                    rr = tiny.tile([I_CAPS, 1], f32, tag="rr", name=f"rr_{rep}_{b}_{it}")
                    nc.scalar.activation(rr[:], lnm[:], ACTF.Exp, scale=-0.5)
                    # rrb[k, q] = rr[k]: fold the normalize scale into the
                    # broadcast matmul's stationary operand (cheaper than
                    # scaling the full masked O)
                    rrb = tiny.tile([I_CAPS, 128], pe_dt, tag="rrb",
                                    name=f"rrb_{rep}_{b}_{it}")
                    nc.vector.tensor_scalar_mul(rrb[:], scratch1[0:I_CAPS, :],
                                                rr[:, 0:1])

                    # ---- broadcast o'_flat across 128 partitions via matmul:
                    #      E[q, f] = sum_k rr[k] om[k, f] = o'[f//16, f%16] ----
                    e_ps = psum.tile([128, IJ], f32, tag="sO", bufs=1,
                                     name=f"Eps_{rep}_{b}_{it}")
                    nc.tensor.matmul(e_ps[:], rrb[:], om_sb[:],
                                     start=True, stop=True)

                    # ---- vT[q, e, i] = sum_j W[128e+q, i*16+j] * o'[i, j] ----
                    vT_sb = tiny.tile([128, DC, I_CAPS], pe_dt, tag="vT",
                                      name=f"vT_{rep}_{b}_{it}")
                    for e in range(DC):
                        wtmp = small.tile([128, IJ], f32, tag="om",
                                          name=f"wtmp_{rep}_{b}_{it}_{e}")
                        w_in = (w_sb[:, e, :] if mybir.dt.size(w_sb.dtype) != 4
                                else w_sb[:, e, :].bitcast(f32))
                        nc.vector.tensor_mul(wtmp[:], w_in, e_ps[:])
                        with nc.allow_low_precision(reason="f32r round on store"):
                            nc.vector.tensor_reduce(
                                vT_sb[:, e, :],
                                wtmp[:].rearrange("q (i j) -> q i j", j=J_DIM),
                                axis=AX.X, op=OP.add)

                    # ---- matmul2 + exp: b[i, nn] = sum_d vT[d, i] uT[d, nn] ----
                    e_sb = e_pool.tile([I_CAPS, n], f32, tag="e", name=f"e_{rep}_{b}_{it}")
                    for k in range(NB):
                        b_ps = psum.tile([I_CAPS, 512], f32, tag="b", bufs=3,
                                         name=f"bps_{rep}_{b}_{it}_{k}")
                        for e in range(DC):
                            nc.tensor.matmul(b_ps[:], vT_sb[:, e, :],
                                             uT_t[:, e, k * 512:(k + 1) * 512],
                                             start=(e == 0), stop=(e == DC - 1))
                        nc.scalar.activation(e_sb[:, k * 512:(k + 1) * 512], b_ps[:],
                                             ACTF.Exp)

                    # ---- transpose e -> eT blocks, softmax over i -> next cT ----
                    cT = cT_pool.tile([128, NC, I_CAPS], pe_dt, tag="cT",
                                      name=f"cT_{rep}_{b}_{it + 1}")
                    z_sb = tiny.tile([128, NC], f32, tag="z", name=f"z_{rep}_{b}_{it}")
                    bpb = 512 // I_CAPS  # transpose blocks per psum bank (16)
                    eT_list = []
                    groups = list(range(0, NC, bpb))
                    for gi in range(0, len(groups), 2):
                        pair = groups[gi:gi + 2]
                        tiles = []
                        for g0 in pair:
                            bw = min(bpb, NC - g0)
                            tiles.append((g0, bw, psum.tile(
                                [128, bw * I_CAPS], f32, tag="tr", bufs=2,
                                name=f"eTps_{rep}_{b}_{it}_{g0}")))
                        for g0, bw, eT_ps in tiles:
                            pack = []
                            for t in range(bw):
                                c = g0 + t
                                pack.append(nc.tensor.matmul(
                                    eT_ps[:, t * I_CAPS:(t + 1) * I_CAPS],
                                    e_sb[:, c * 128:(c + 1) * 128],
                                    ident[0:I_CAPS, 0:I_CAPS],
                                    is_transpose=True,
                                    start=(t == 0), stop=(t == bw - 1)))
                            chain(pack)
                        for g0, bw, eT_ps in tiles:
                            eT_list.append((g0, bw, eT_ps))
                    r_sb = tiny.tile([128, NC], f32, tag="r", name=f"r_{rep}_{b}_{it}")
                    for g0, bw, eT_ps in eT_list:
                        hw2 = (bw + 1) // 2
                        for h0 in range(0, bw, hw2):
                            hb = min(hw2, bw - h0)
                            ev = eT_ps[:, h0 * I_CAPS:(h0 + hb) * I_CAPS]
                            ev = ev.rearrange("q (c i) -> q c i", i=I_CAPS)
                            nc.vector.tensor_reduce(
                                z_sb[:, g0 + h0:g0 + h0 + hb], ev,
                                axis=AX.X, op=OP.add)
                            nc.vector.reciprocal(
                                r_sb[:, g0 + h0:g0 + h0 + hb],
                                z_sb[:, g0 + h0:g0 + h0 + hb])
                            rb = r_sb[:, g0 + h0:g0 + h0 + hb]
                            rb = rb.unsqueeze(2).broadcast_to([128, hb, I_CAPS])
                            nc.vector.tensor_mul(
                                cT[:, g0 + h0:g0 + h0 + hb, :], ev, rb)
                else:
                    # ---- squash: out = sqrt(s2)/(0.5+s2) * o, s2 = nrm + EPS.
                    # scl = exp(0.5*ln(nrm+EPS) - ln(nrm+EPS+0.5)); Ln/Exp only
                    # (same ACT table set as the softmax Exp). ----
                    lns = tiny.tile([I_CAPS, 1], f32, tag="lns", name=f"lns_{rep}_{b}")
                    nc.scalar.activation(lns[:], nrm[:], ACTF.Ln,
                                         bias=eps_ap[:, 0:1])
                    lnden = tiny.tile([I_CAPS, 1], f32, tag="lnden",
                                      name=f"lnden_{rep}_{b}")
                    nc.scalar.activation(lnden[:], nrm[:], ACTF.Ln,
                                         bias=eps_half_ap[:, 0:1])
                    scl_ln = tiny.tile([I_CAPS, 1], f32, tag="scl_ln",
                                       name=f"scl_ln_{rep}_{b}")
                    nc.vector.scalar_tensor_tensor(
                        scl_ln[:], lns[:], 0.5, lnden[:],
                        op0=OP.mult, op1=OP.subtract)
                    scl = tiny.tile([I_CAPS, 1], f32, tag="scl", name=f"scl_{rep}_{b}")
                    nc.scalar.activation(scl[:], scl_ln[:], ACTF.Exp)
                    o_out = tiny.tile([I_CAPS, J_DIM], f32, tag="oout",
                                      name=f"oout_{rep}_{b}")
                    nc.vector.tensor_scalar_mul(o_out[:], o_sb[:], scl[:, 0:1])
                    nc.sync.dma_start(out_dram[b], o_out[:])


_NC_CACHE = {}


def _get_nc():
    if "nc" not in _NC_CACHE:
        _NC_CACHE["nc"] = build_nc()
    return _NC_CACHE["nc"]


def kernel(u_vecs: np.ndarray, W: np.ndarray) -> np.ndarray:
    from concourse.bass_utils import run_bass_kernel_spmd

    u_vecs = np.ascontiguousarray(u_vecs, dtype=np.float32)
    W = np.ascontiguousarray(W, dtype=np.float32)
    b_loc = B // N_CORES
    nc = _get_nc()
    in_maps = [
        {"u": u_vecs[i * b_loc:(i + 1) * b_loc], "w": W}
        for i in range(N_CORES)
    ]
    res = run_bass_kernel_spmd(nc, in_maps, core_ids=list(range(N_CORES)))
    return np.concatenate([r["out"] for r in res.results], axis=0)


def _kernel_body_pair(nc, tc, mybir, b_loc, n, d, NC, DC, NB, IJ, f32, pe_dt,
                      AX, OP, ACTF, u_dram, w_dram, out_dram,
                      u_pool, uT_pool, cT_pool, e_pool, small, tiny, psum,
                      ident, ident_r, scratch1, c0, mask, w_sb, rep,
                      eps_ap, eps_half_ap):
    """Two batches processed together: their [32, *] tensors stack on PSUM
    column groups (tile_position=(0, 32) for the odd batch), halving the
    count/latency of the small-op chains and running softmax/extract ops at
    2x lane efficiency."""
    P = 2 * I_CAPS  # 64

    assert b_loc % 2 == 0
    for pb in range(b_loc // 2):
        u_ts, uT_ts = [], []
        for bb in range(2):
            b = 2 * pb + bb
            u_t = u_pool.tile([128, NC, d], pe_dt, tag="u",
                              name=f"u_{rep}_{b}")
            nc.gpsimd.dma_start(
                u_t[:], u_dram[b].rearrange("(p c) dd -> p c dd", c=NC))
            uT_t = uT_pool.tile([128, DC, n], pe_dt, tag="uT",
                                name=f"uT_{rep}_{b}")
            cp_flip = 0
            for e in range(DC):
                for cg in range(0, NC, 4):
                    tr_ps = psum.tile([128, 512], pe_dt, tag="tr", bufs=2,
                                      name=f"trps_{rep}_{b}_{e}_{cg}")
                    with tc.tile_critical():
                        for k in range(4):
                            c = cg + k
                            nc.tensor.matmul(
                                tr_ps[:, k * 128:(k + 1) * 128],
                                u_t[:, c, e * 128:(e + 1) * 128],
                                ident_r[:],
                                is_transpose=True, start=(k == 0), stop=(k == 3))
                    dst = uT_t[:, e, cg * 128:(cg + 4) * 128]
                    if cp_flip % 2 == 0:
                        nc.vector.tensor_copy(dst, tr_ps[:])
                    else:
                        nc.scalar.copy(dst, tr_ps[:])
                    cp_flip += 1
            u_ts.append(u_t)
            uT_ts.append(uT_t)

        cT = None
        for it in range(ROUTINGS):
            # ---- matmul1 pair: s[32bb+i, dd] = sum_n c_bb[i, n] u_bb[n, dd].
            # One PSUM bank, col-group 0 for even batch, col-group 1 for odd
            # (tile_position). Critical: odd group start=True clears whole-bank
            # has_written bits, so groups must not interleave on the PE. ----
            s_ps = psum.tile([P, d], f32, tag="s", bufs=1,
                             name=f"sps_{rep}_{pb}_{it}")
            for bb in range(2):
                out_sl = s_ps[bb * I_CAPS:(bb + 1) * I_CAPS, :]
                for c in range(NC):
                    lhs1 = (c0[:, bb * I_CAPS:(bb + 1) * I_CAPS]
                            if cT is None
                            else cT[:, c, bb * I_CAPS:(bb + 1) * I_CAPS])
                    nc.tensor.matmul(
                        out_sl, lhs1, u_ts[bb][:, c, :],
                        start=(c == 0), stop=(c == NC - 1),
                        tile_position=(0, bb * I_CAPS))
            s_sb = small.tile([P, d], pe_dt, tag="s_sb",
                              name=f"ssb_{rep}_{pb}_{it}")
            nc.vector.tensor_copy(s_sb[:], s_ps[:])

            # ---- sT[q, 64e + 32bb + i] = s[32bb + i, 128e+q] ----
            sT_ps = psum.tile([128, DC * P], pe_dt, tag="tr", bufs=2,
                              name=f"sTps_{rep}_{pb}_{it}")
            with tc.tile_critical():
                for e in range(DC):
                    nc.tensor.matmul(
                        sT_ps[:, e * P:(e + 1) * P],
                        s_sb[:, e * 128:(e + 1) * 128],
                        ident_r[0:P, 0:P],
                        is_transpose=True, start=(e == 0), stop=(e == DC - 1))
            sT_sb = small.tile([128, DC * P], pe_dt, tag="sT_sb",
                               name=f"sTsb_{rep}_{pb}_{it}")
            nc.scalar.copy(sT_sb[:], sT_ps[:])

            # ---- O_full pair = S @ W  [64, 512] ----
            o_ps = psum.tile([P, IJ], f32, tag="O", bufs=1,
                             name=f"Ops_{rep}_{pb}_{it}")
            for bb in range(2):
                out_sl = o_ps[bb * I_CAPS:(bb + 1) * I_CAPS, :]
                for e in range(DC):
                    nc.tensor.matmul(
                        out_sl,
                        sT_sb[:, e * P + bb * I_CAPS:
                              e * P + (bb + 1) * I_CAPS],
                        w_sb[:, e, :],
                        start=(e == 0), stop=(e == DC - 1),
                        tile_position=(0, bb * I_CAPS))

            # ---- extract o[32bb+i, j] = O_full[32bb+i, i*16+j] ----
            om_sb = small.tile([P, IJ], pe_dt, tag="om",
                               name=f"om_{rep}_{pb}_{it}")
            nc.vector.tensor_mul(om_sb[:], o_ps[:], mask[:])
            om_f = (om_sb[:] if mybir.dt.size(om_sb.dtype) != 4
                    else om_sb[:].bitcast(f32))
            o_sb = tiny.tile([P, J_DIM], f32, tag="o", name=f"o_{rep}_{pb}_{it}")
            nc.vector.tensor_reduce(
                o_sb[:], om_f.rearrange("p (i j) -> p j i", j=J_DIM),
                axis=AX.X, op=OP.add)

            sq = tiny.tile([P, J_DIM], f32, tag="sq", name=f"sq_{rep}_{pb}_{it}")
            nrm = tiny.tile([P, 1], f32, tag="nrm", name=f"nrm_{rep}_{pb}_{it}")
            nc.scalar.activation(sq[:], o_sb[:], ACTF.Square, accum_out=nrm[:])

            if it < ROUTINGS - 1:
                lnm = tiny.tile([P, 1], f32, tag="lnm",
                                name=f"lnm_{rep}_{pb}_{it}")
                nc.scalar.activation(lnm[:], nrm[:], ACTF.Ln)
                rr = tiny.tile([P, 1], f32, tag="rr", name=f"rr_{rep}_{pb}_{it}")
                nc.scalar.activation(rr[:], lnm[:], ACTF.Exp, scale=-0.5)
                rrb = tiny.tile([P, 128], pe_dt, tag="rrb",
                                name=f"rrb_{rep}_{pb}_{it}")
                nc.vector.tensor_scalar_mul(rrb[:], scratch1[0:P, :], rr[:, 0:1])

                # E[q, 512bb + f] = sum_i rr[32bb+i] om[32bb+i, f] (2 banks)
                e_ps = psum.tile([128, 2 * IJ], f32, tag="E", bufs=1,
                                 name=f"Eps_{rep}_{pb}_{it}")
                for bb in range(2):
                    nc.tensor.matmul(
                        e_ps[:, bb * IJ:(bb + 1) * IJ],
                        rrb[bb * I_CAPS:(bb + 1) * I_CAPS, :],
                        om_sb[bb * I_CAPS:(bb + 1) * I_CAPS, :],
                        start=True, stop=True)

                # vT[q, e, bb, i] = sum_j W[128e+q, i*16+j] o_bb'[i, j]
                vT_sb = tiny.tile([128, DC, 2, I_CAPS], pe_dt, tag="vT",
                                  name=f"vT_{rep}_{pb}_{it}")
                for e in range(DC):
                    wtmp = small.tile([128, 2 * IJ], f32, tag="om",
                                      name=f"wtmp_{rep}_{pb}_{it}_{e}")
                    w_in = (w_sb[:, e, :] if mybir.dt.size(w_sb.dtype) != 4
                            else w_sb[:, e, :].bitcast(f32))
                    w_in = w_in.unsqueeze(1).broadcast_to([128, 2, IJ])
                    nc.vector.tensor_mul(
                        wtmp[:].rearrange("q (bb f) -> q bb f", bb=2),
                        w_in, e_ps[:].rearrange("q (bb f) -> q bb f", bb=2))
                    with nc.allow_low_precision(reason="f32r round on store"):
                        nc.vector.tensor_reduce(
                            vT_sb[:, e, :, :],
                            wtmp[:].rearrange("q (bb i j) -> q bb i j",
                                              bb=2, j=J_DIM),
                            axis=AX.X, op=OP.add)

                # ---- matmul2 pair + exp ----
                e_sb = e_pool.tile([P, n], f32, tag="e", bufs=1,
                                   name=f"e_{rep}_{pb}_{it}")
                for k in range(NB):
                    b_ps = psum.tile([P, 512], f32, tag="b", bufs=2,
                                     name=f"bps_{rep}_{pb}_{it}_{k}")
                    for bb in range(2):
                        out_sl = b_ps[bb * I_CAPS:(bb + 1) * I_CAPS, :]
                        for e in range(DC):
                            nc.tensor.matmul(
                                out_sl, vT_sb[:, e, bb, :],
                                uT_ts[bb][:, e, k * 512:(k + 1) * 512],
                                start=(e == 0), stop=(e == DC - 1),
                                tile_position=(0, bb * I_CAPS))
                    nc.scalar.activation(e_sb[:, k * 512:(k + 1) * 512],
                                         b_ps[:], ACTF.Exp)

                # ---- transpose e pair -> softmax over i -> next cT ----
                cT = cT_pool.tile([128, NC, P], pe_dt, tag="cT",
                                  name=f"cT_{rep}_{pb}_{it + 1}")
                z_sb = tiny.tile([128, NC, 2], f32, tag="z",
                                 name=f"z_{rep}_{pb}_{it}")
                r_sb = tiny.tile([128, NC, 2], f32, tag="r",
                                 name=f"r_{rep}_{pb}_{it}")
                bpb = 512 // P  # 8 transpose blocks per psum bank
                # complete softmax per bank so each eT psum tile releases
                # before the next bank's transposes need a slot
                for g0 in range(0, NC, bpb):
                    bw = min(bpb, NC - g0)
                    eT_ps = psum.tile([128, bw * P], f32, tag="tr", bufs=2,
                                      name=f"eTps_{rep}_{pb}_{it}_{g0}")
                    with tc.tile_critical():
                        for t in range(bw):
                            c = g0 + t
                            nc.tensor.matmul(
                                eT_ps[:, t * P:(t + 1) * P],
                                e_sb[:, c * 128:(c + 1) * 128],
                                ident[0:P, 0:P],
                                is_transpose=True,
                                start=(t == 0), stop=(t == bw - 1))
                    nc.vector.tensor_reduce(
                        z_sb[:, g0:g0 + bw, :],
                        eT_ps[:].rearrange("q (c bb i) -> q c bb i",
                                           bb=2, i=I_CAPS),
                        axis=AX.X, op=OP.add)
                    nc.vector.reciprocal(r_sb[:, g0:g0 + bw, :],
                                         z_sb[:, g0:g0 + bw, :])
                    rb = r_sb[:, g0:g0 + bw, :]
                    rb = rb.unsqueeze(3).broadcast_to([128, bw, 2, I_CAPS])
                    nc.vector.tensor_mul(
                        cT[:, g0:g0 + bw, :].rearrange(
                            "q c (bb i) -> q c bb i", bb=2),
                        eT_ps[:].rearrange("q (c bb i) -> q c bb i",
                                           bb=2, i=I_CAPS), rb)
            else:
                # ---- squash pair + output ----
                lns = tiny.tile([P, 1], f32, tag="lns", name=f"lns_{rep}_{pb}")
                nc.scalar.activation(lns[:], nrm[:], ACTF.Ln,
                                     bias=eps_ap[:, 0:1])
                lnden = tiny.tile([P, 1], f32, tag="lnden",
                                  name=f"lnden_{rep}_{pb}")
                nc.scalar.activation(lnden[:], nrm[:], ACTF.Ln,
                                     bias=eps_half_ap[:, 0:1])
                scl_ln = tiny.tile([P, 1], f32, tag="scl_ln",
                                   name=f"scl_ln_{rep}_{pb}")
                nc.vector.scalar_tensor_tensor(
                    scl_ln[:], lns[:], 0.5, lnden[:],
                    op0=OP.mult, op1=OP.subtract)
                scl = tiny.tile([P, 1], f32, tag="scl", name=f"scl_{rep}_{pb}")
                nc.scalar.activation(scl[:], scl_ln[:], ACTF.Exp)
                o_out = tiny.tile([P, J_DIM], f32, tag="oout",
                                  name=f"oout_{rep}_{pb}")
                nc.vector.tensor_scalar_mul(o_out[:], o_sb[:], scl[:, 0:1])
                for bb in range(2):
                    nc.sync.dma_start(
                        out_dram[2 * pb + bb],
                        o_out[bb * I_CAPS:(bb + 1) * I_CAPS, :])
